# revision 54
# baseline (speedup 1.0000x reference)
"""Trainium2 Bass kernel for a 13-layer causal dilated conv stack with gating.

Model (per reference):
    Wx_f = 13 causal dilated convs (K=2, dilation 2^i) over x with Wf
    Wx_g = same with Wg
    out  = tanh(Wx_f + h@Vf) * sigmoid(Wx_g + h@Vg)

Shapes: x (16, 8192, 64) f32, h (16, 64), Wf/Wg (13, 2, 64, 64), Vf/Vg (64, 64).

Strategy:
  - Data-parallel over batch: 2 batch elements per core on 8 cores, no
    collectives.
  - On-chip layout [128 partitions = (b*64 + c), T free]: both local batch
    elements' channels stacked on the partition axis. The host pre-transposes
    x into this channel-major layout (and transposes the output back), so the
    device does no layout changes at all.
  - Each conv layer is, per 512-token tile, two accumulating PE matmuls (one
    per tap) with block-diagonal kron(I2, W[tap]) stationary weights -> full
    128-wide PE utilization.
  - Causality: activation buffers carry a 256-column zero margin covering
    dilations < 512; for d >= 512 the boundary is tile-aligned and the tap-0
    matmul is simply skipped on the first d/512 tiles.
  - fp32r matmuls (PE full-rate fp32 for moving dim >= 256). fp32r is fp32
    rounded to an 11-bit mantissa, left-aligned in the 32-bit word (low 12
    bits zero). x and the packed weights are converted to fp32r BITS on the
    host and DMA'd straight into fp32r tensors; on-device activations are
    rounded by the f32r-typed DVE/ACT drain copies.
  - h@V bias is computed on-device with kron(I2, V) and fused into the
    tanh/sigmoid activations via the ScalarE bias port.
  - PSUM->SBUF drain copies alternate DVE/DVE/ACT so neither engine
    bottlenecks the PE (scratch stays DVE-only for the output DMAs).
  - PE matmuls and HWDGE DMA descriptors only support a single sync wait and
    Tile's wait pass is not transitively minimal. The kernel therefore (a)
    warms the PE's vector clock with one tiny matmul per input-DMA lane so
    real matmuls never re-wait DMA lanes, and (b) keeps each of the 3 input
    / 4 output DMAs on its own HWDGE lane with single-engine dependencies.
"""

import sys

import numpy as np

for _p in ("/opt/trn_rl_repo",):
    if _p not in sys.path:
        sys.path.append(_p)

B, T, C = 16, 8192, 64
K = 2
NUM_LAYERS = 13
N_CORES = 8
BPC = B // N_CORES          # batch elements per core
P = 2 * C                   # partitions used: (b, c) pairs
NTAP_TILE = 512             # tokens per matmul tile
NT = T // NTAP_TILE         # matmul tiles per layer
MARGIN = 256                # causal zero margin (covers dilations < 512)
NW = 2 * NUM_LAYERS * K     # packed conv weight count
NQ = 8                      # output DMA chunks
QT = T // NQ
NXC = 8                     # input x DMA chunks
XCT = T // NXC
W_HEAD = 8                  # weight tiles in the head DMA (first 4 layers run)

# layer execution schedule (branch, layer), chosen so BOTH branches' layer 0
# run during the x-chunk-paced startup (g-l0 parks its output in scratch,
# which is otherwise idle until f-l12), and weights are packed in this order
# so the head DMA covers exactly the first layers.
SCHED = ([("f", 0), ("g", 0)] + [("f", l) for l in range(1, 12)] +
         [("g", 1), ("f", 12)] + [("g", l) for l in range(2, 13)])

_PROGRAM_CACHE = {}


def fp32r_bits(a):
    """Round f32 to fp32r (11-bit mantissa, RNE), low 12 bits zeroed."""
    u = np.ascontiguousarray(a, dtype=np.float32).view(np.uint32)
    keep = u >> np.uint32(12)
    low = u & np.uint32(0xFFF)
    rup = (low > 0x800) | ((low == 0x800) & ((keep & np.uint32(1)) == 1))
    return ((keep + rup.astype(np.uint32)) << np.uint32(12)).view(np.float32)


def _build_program():
    import concourse.bacc as bacc
    import concourse.tile as tile
    from concourse import mybir

    f32 = mybir.dt.float32
    f32r = mybir.dt.float32r
    AF = mybir.ActivationFunctionType

    # Bacc (not bare Bass): its compile() legalizes multi-wait instructions
    # into event-semaphore chains (TRN2 allows 1 wait per instruction).
    nc = bacc.Bacc("TRN2", target_bir_lowering=False, debug=False)

    consts = nc.dram_tensor("consts", [P, 2 * P + 1], f32,
                            kind="ExternalInput").ap()
    xin = nc.dram_tensor("xr", [P, T], f32r, kind="ExternalInput").ap()
    wconv = nc.dram_tensor("wr", [P, NW * P], f32r, kind="ExternalInput").ap()
    outs = [nc.dram_tensor(f"out_q{q}", [P, QT], f32,
                           kind="ExternalOutput").ap() for q in range(NQ)]

    with tile.TileContext(nc) as tc:
        with (
            tc.tile_pool(name="persist", bufs=1) as persist,
            tc.tile_pool(name="epool", bufs=4) as epool,
            tc.tile_pool(name="mpsum", bufs=8, space="PSUM") as mpsum,
        ):
            # ---- persistent buffers ---------------------------------------
            scratch = persist.tile([P, MARGIN + T], f32r, name="scratch",
                                   tag="scratch")
            call = persist.tile([P, 2 * P + 1], f32, name="call", tag="call")
            x0 = persist.tile([P, MARGIN + T], f32r, name="x0", tag="x0")
            bufA = persist.tile([P, MARGIN + T], f32r, name="bufA", tag="bufA")
            bufB = persist.tile([P, MARGIN + T], f32r, name="bufB", tag="bufB")
            wall = persist.tile([P, NW * P], f32r, name="wall", tag="wall")
            for buf in (x0, bufA, bufB, scratch):
                # f32 view: a memset of an f32r-typed AP is not valid ISA
                nc.vector.memset(buf.bitcast(f32)[:, 0:MARGIN], 0.0)
            scratch32 = scratch.bitcast(f32)

            # ---- input DMAs, ordered by first consumption -----------------
            # weight head (f-branch layers 0-5) -> x chunks -> weight rest,
            # so layer-0 compute starts ~3us in and later layers never stall.
            nc.sync.dma_start(out=call, in_=consts)
            nc.sync.dma_start(out=wall[:, 0:W_HEAD * P],
                              in_=wconv[:, 0:W_HEAD * P])
            for xc in range(NXC):
                nc.sync.dma_start(
                    out=x0[:, MARGIN + xc * XCT:MARGIN + (xc + 1) * XCT],
                    in_=xin[:, xc * XCT:(xc + 1) * XCT])
            nc.sync.dma_start(out=wall[:, W_HEAD * P:],
                              in_=wconv[:, W_HEAD * P:])

            # ---- PE lane warm-ups + h @ V biases --------------------------
            # 4 single-shot matmuls into disjoint columns of one PSUM tile:
            # two consume the x/w DMA lanes (results unused), then the two
            # bias projections (their lane comes via their own operands).
            # lives in the conv-psum ring: only needed in the first few us,
            # so it must not pin a PSUM bank for the whole kernel
            bias_ps = mpsum.tile([P, 8], f32, name="bias_ps", tag="mp")
            x0w = x0.bitcast(f32)[:, MARGIN:MARGIN + 1]
            ww = wall.bitcast(f32)[:, 0:1]
            nc.tensor.matmul(bias_ps[0:1, 4:5], lhsT=x0w, rhs=x0w,
                             start=True, stop=True)
            nc.tensor.matmul(bias_ps[0:1, 5:6], lhsT=ww, rhs=ww,
                             start=True, stop=True)
            h_t = call[:, 2 * P:2 * P + 1]
            bias = []
            for i in range(2):
                nc.tensor.matmul(bias_ps[:, i:i + 1],
                                 lhsT=call[:, i * P:(i + 1) * P],
                                 rhs=h_t, start=True, stop=True)
                bias_sb = persist.tile([P, 1], f32, name=f"bias{i}",
                                       tag=f"bias{i}")
                nc.vector.tensor_copy(bias_sb, bias_ps[:, i:i + 1])
                bias.append(bias_sb)

            # ---- conv stacks ----------------------------------------------
            drain_rr = [0]

            def conv_layer(src, layer, br, dst=None, fuse=None):
                d = 2 ** layer
                base = 2 * SCHED.index((br, layer))
                w0r = wall[:, base * P:(base + 1) * P]
                w1r = wall[:, (base + 1) * P:(base + 2) * P]
                for j in range(NT):
                    t0 = j * NTAP_TILE
                    # d < 512 boundary reads dip into the zero margin;
                    # d >= 512 boundaries are tile-aligned -> tap 0 skipped.
                    has_tap0 = t0 + NTAP_TILE > d
                    ps = mpsum.tile([P, NTAP_TILE], f32,
                                    name=f"ps_{br}{layer}_{j}", tag="mp")
                    nc.tensor.matmul(
                        ps, lhsT=w1r,
                        rhs=src[:, MARGIN + t0:MARGIN + t0 + NTAP_TILE],
                        start=True, stop=not has_tap0)
                    if has_tap0:
                        o0 = MARGIN + t0 - d
                        nc.tensor.matmul(
                            ps, lhsT=w0r, rhs=src[:, o0:o0 + NTAP_TILE],
                            start=False, stop=True)
                    if fuse is None:
                        dslice = dst[:, MARGIN + t0:MARGIN + t0 + NTAP_TILE]
                        # alternate drains DVE/ACT so neither engine gates PE
                        if drain_rr[0] % 2 == 1:
                            nc.scalar.copy(dslice, ps)
                        else:
                            nc.vector.tensor_copy(dslice, ps)
                        drain_rr[0] += 1
                    else:
                        fuse(j, ps)

            # f-l12's drain IS the tanh (ScalarE, fused bias) straight into
            # scratch (g-l0's parked output is dead once g-l1 has read it),
            # keeping the final g-layer's epilogue off the ACT critical path.
            def tanh_drain(j, ps_f):
                t0 = MARGIN + j * NTAP_TILE
                nc.scalar.activation(scratch[:, t0:t0 + NTAP_TILE], ps_f,
                                     AF.Tanh, bias=bias[0])

            # g-l12 fused with the gating epilogue
            def epilogue(j, ps_g):
                t0 = MARGIN + j * NTAP_TILE
                sig = epool.tile([P, NTAP_TILE], f32, name=f"sig{j}", tag="sig")
                nc.scalar.activation(sig, ps_g, AF.Sigmoid, bias=bias[1])
                # scratch[j] holds tanh(f); the gated output overwrites it
                # and is DMA'd out from there. Read via the f32 view (fp32r
                # bits are valid rounded-fp32 bits); write stays f32r-typed.
                # Early tiles multiply on the otherwise-idle GpSimd (off the
                # critical path while the PE still runs); DVE keeps only the
                # tail tiles, halving the post-PE epilogue backlog.
                eng = nc.gpsimd if j < 10 else nc.vector
                eng.tensor_mul(scratch[:, t0:t0 + NTAP_TILE],
                               scratch32[:, t0:t0 + NTAP_TILE], sig)

            # buffer rotation per SCHED:
            #   f: x0 -> A -> B -> A ... (f-l12 reads B, tanh -> scratch)
            #   g: x0 -> scratch -> A -> B ... (g-l12 reads A, epilogue)
            cur = {"f": x0, "g": x0}
            for br, layer in SCHED:
                if (br, layer) == ("f", 12):
                    conv_layer(cur["f"], layer, br, fuse=tanh_drain)
                elif (br, layer) == ("g", 12):
                    conv_layer(cur["g"], layer, br, fuse=epilogue)
                else:
                    if br == "f":
                        dst = bufA if layer % 2 == 0 else bufB
                    else:
                        dst = scratch if layer == 0 else \
                            (bufA if layer % 2 == 1 else bufB)
                    conv_layer(cur[br], layer, br, dst=dst)
                    cur[br] = dst

            # ---- store output (channel-major; host restores [b,t,c]) ------
            for q in range(NQ):
                nc.sync.dma_start(
                    out=outs[q],
                    in_=scratch32[:, MARGIN + q * QT:MARGIN + (q + 1) * QT])

    nc.compile()
    return nc


def get_program():
    if "nc" not in _PROGRAM_CACHE:
        _PROGRAM_CACHE["nc"] = _build_program()
    return _PROGRAM_CACHE["nc"]


def make_in_maps(x, h, Wf, Wg, Vf, Vg):
    x = np.asarray(x, dtype=np.float32)
    h = np.asarray(h, dtype=np.float32)
    eye2 = np.eye(2, dtype=np.float32)
    # SCHED-ordered [branch, layer, tap] -> kron(I2, W[tap]) as lhsT
    # [K=(b,cin), M=(b,cout)]
    Wn = {"f": np.asarray(Wf, dtype=np.float32),
          "g": np.asarray(Wg, dtype=np.float32)}
    wpack = np.zeros((NW, P, P), dtype=np.float32)
    for pos, (br, layer) in enumerate(SCHED):
        for tap in range(K):
            wpack[2 * pos + tap] = np.kron(eye2, Wn[br][layer, tap])
    # wall[p, i*P + m] = wpack[i, p, m], pre-rounded to fp32r bits
    wcols = fp32r_bits(wpack.transpose(1, 0, 2).reshape(P, NW * P))
    vcat = np.concatenate(
        [np.kron(eye2, np.asarray(V, dtype=np.float32)) for V in (Vf, Vg)],
        axis=1)  # [128, 256]

    in_maps = []
    for core in range(N_CORES):
        sl = slice(core * BPC, (core + 1) * BPC)
        xcm = fp32r_bits(x[sl].transpose(0, 2, 1).reshape(P, T))  # [(b,c), t]
        consts = np.ascontiguousarray(
            np.concatenate([vcat, h[sl].reshape(P, 1)], axis=1))
        in_maps.append({"consts": consts, "xr": xcm, "wr": wcols})
    return in_maps


def assemble_output(results):
    full = np.empty((B, T, C), dtype=np.float32)
    for core, r in enumerate(results):
        cm = np.concatenate([r[f"out_q{q}"] for q in range(NQ)], axis=1)
        full[core * BPC:(core + 1) * BPC] = \
            cm.reshape(BPC, C, T).transpose(0, 2, 1)
    return full


def kernel(x, h, Wf, Wg, Vf, Vg):
    from concourse import bass_utils

    nc = get_program()
    in_maps = make_in_maps(x, h, Wf, Wg, Vf, Vg)
    res = bass_utils.run_bass_kernel_spmd(nc, in_maps,
                                          core_ids=list(range(N_CORES)))
    return assemble_output(res.results)


# revision 55
# speedup vs baseline: 1.0324x; 1.0324x over previous
"""Trainium2 Bass kernel for a 13-layer causal dilated conv stack with gating.

Model (per reference):
    Wx_f = 13 causal dilated convs (K=2, dilation 2^i) over x with Wf
    Wx_g = same with Wg
    out  = tanh(Wx_f + h@Vf) * sigmoid(Wx_g + h@Vg)

Shapes: x (16, 8192, 64) f32, h (16, 64), Wf/Wg (13, 2, 64, 64), Vf/Vg (64, 64).

Strategy:
  - Data-parallel over batch: 2 batch elements per core on 8 cores, no
    collectives.
  - On-chip layout [128 partitions = (b*64 + c), T free]: both local batch
    elements' channels stacked on the partition axis. The host pre-transposes
    x into this channel-major layout (and transposes the output back), so the
    device does no layout changes at all.
  - Each conv layer is, per 512-token tile, two accumulating PE matmuls (one
    per tap) with block-diagonal kron(I2, W[tap]) stationary weights -> full
    128-wide PE utilization.
  - Causality: activation buffers carry a 256-column zero margin covering
    dilations < 512; for d >= 512 the boundary is tile-aligned and the tap-0
    matmul is simply skipped on the first d/512 tiles.
  - fp32r matmuls (PE full-rate fp32 for moving dim >= 256). fp32r is fp32
    rounded to an 11-bit mantissa, left-aligned in the 32-bit word (low 12
    bits zero). x and the packed weights are converted to fp32r BITS on the
    host and DMA'd straight into fp32r tensors; on-device activations are
    rounded by the f32r-typed DVE/ACT drain copies.
  - h@V bias is computed on-device with kron(I2, V) and fused into the
    tanh/sigmoid activations via the ScalarE bias port.
  - PSUM->SBUF drain copies alternate DVE/DVE/ACT so neither engine
    bottlenecks the PE (scratch stays DVE-only for the output DMAs).
  - PE matmuls and HWDGE DMA descriptors only support a single sync wait and
    Tile's wait pass is not transitively minimal. The kernel therefore (a)
    warms the PE's vector clock with one tiny matmul per input-DMA lane so
    real matmuls never re-wait DMA lanes, and (b) keeps each of the 3 input
    / 4 output DMAs on its own HWDGE lane with single-engine dependencies.
"""

import sys

import numpy as np

for _p in ("/opt/trn_rl_repo",):
    if _p not in sys.path:
        sys.path.append(_p)

B, T, C = 16, 8192, 64
K = 2
NUM_LAYERS = 13
N_CORES = 8
BPC = B // N_CORES          # batch elements per core
P = 2 * C                   # partitions used: (b, c) pairs
NTAP_TILE = 512             # tokens per matmul tile
NT = T // NTAP_TILE         # matmul tiles per layer
MARGIN = 256                # causal zero margin (covers dilations < 512)
NW = 2 * NUM_LAYERS * K     # packed conv weight count
NQ = 8                      # output DMA chunks
QT = T // NQ
NXC = 8                     # input x DMA chunks
XCT = T // NXC
W_HEAD = 8                  # weight tiles in the head DMA (first 4 layers run)

# layer execution schedule (branch, layer), chosen so BOTH branches' layer 0
# run during the x-chunk-paced startup (g-l0 parks its output in scratch,
# which is otherwise idle until f-l12), and weights are packed in this order
# so the head DMA covers exactly the first layers.
SCHED = ([("f", 0), ("g", 0)] + [("f", l) for l in range(1, 12)] +
         [("g", 1), ("f", 12)] + [("g", l) for l in range(2, 13)])

_PROGRAM_CACHE = {}


def fp32r_bits(a):
    """Round f32 to fp32r (11-bit mantissa, RNE), low 12 bits zeroed."""
    u = np.ascontiguousarray(a, dtype=np.float32).view(np.uint32)
    keep = u >> np.uint32(12)
    low = u & np.uint32(0xFFF)
    rup = (low > 0x800) | ((low == 0x800) & ((keep & np.uint32(1)) == 1))
    return ((keep + rup.astype(np.uint32)) << np.uint32(12)).view(np.float32)


def _build_program():
    import concourse.bacc as bacc
    import concourse.tile as tile
    from concourse import mybir

    f32 = mybir.dt.float32
    f32r = mybir.dt.float32r
    AF = mybir.ActivationFunctionType

    # Bacc (not bare Bass): its compile() legalizes multi-wait instructions
    # into event-semaphore chains (TRN2 allows 1 wait per instruction).
    nc = bacc.Bacc("TRN2", target_bir_lowering=False, debug=False)

    consts = nc.dram_tensor("consts", [P, 2 * P + 1], f32,
                            kind="ExternalInput").ap()
    xin = nc.dram_tensor("xr", [P, T], f32r, kind="ExternalInput").ap()
    wconv = nc.dram_tensor("wr", [P, NW * P], f32r, kind="ExternalInput").ap()
    outs = [nc.dram_tensor(f"out_q{q}", [P, QT], f32,
                           kind="ExternalOutput").ap() for q in range(NQ)]

    with tile.TileContext(nc) as tc:
        with (
            tc.tile_pool(name="persist", bufs=1) as persist,
            tc.tile_pool(name="epool", bufs=4) as epool,
            tc.tile_pool(name="mpsum", bufs=8, space="PSUM") as mpsum,
        ):
            # ---- persistent buffers ---------------------------------------
            scratch = persist.tile([P, MARGIN + T], f32r, name="scratch",
                                   tag="scratch")
            call = persist.tile([P, 2 * P + 1], f32, name="call", tag="call")
            x0 = persist.tile([P, MARGIN + T], f32r, name="x0", tag="x0")
            bufA = persist.tile([P, MARGIN + T], f32r, name="bufA", tag="bufA")
            bufB = persist.tile([P, MARGIN + T], f32r, name="bufB", tag="bufB")
            wall = persist.tile([P, NW * P], f32r, name="wall", tag="wall")
            for buf in (x0, bufA, bufB, scratch):
                # f32 view: a memset of an f32r-typed AP is not valid ISA
                nc.vector.memset(buf.bitcast(f32)[:, 0:MARGIN], 0.0)
            scratch32 = scratch.bitcast(f32)

            # ---- input DMAs, ordered by first consumption -----------------
            # weight head (f-branch layers 0-5) -> x chunks -> weight rest,
            # so layer-0 compute starts ~3us in and later layers never stall.
            nc.sync.dma_start(out=call, in_=consts)
            nc.sync.dma_start(out=wall[:, 0:W_HEAD * P],
                              in_=wconv[:, 0:W_HEAD * P])
            for xc in range(NXC):
                nc.sync.dma_start(
                    out=x0[:, MARGIN + xc * XCT:MARGIN + (xc + 1) * XCT],
                    in_=xin[:, xc * XCT:(xc + 1) * XCT])
            nc.sync.dma_start(out=wall[:, W_HEAD * P:],
                              in_=wconv[:, W_HEAD * P:])

            # ---- PE lane warm-ups + h @ V biases --------------------------
            # 4 single-shot matmuls into disjoint columns of one PSUM tile:
            # two consume the x/w DMA lanes (results unused), then the two
            # bias projections (their lane comes via their own operands).
            # lives in the conv-psum ring: only needed in the first few us,
            # so it must not pin a PSUM bank for the whole kernel
            bias_ps = mpsum.tile([P, 8], f32, name="bias_ps", tag="mp")
            x0w = x0.bitcast(f32)[:, MARGIN:MARGIN + 1]
            ww = wall.bitcast(f32)[:, 0:1]
            nc.tensor.matmul(bias_ps[0:1, 4:5], lhsT=x0w, rhs=x0w,
                             start=True, stop=True)
            nc.tensor.matmul(bias_ps[0:1, 5:6], lhsT=ww, rhs=ww,
                             start=True, stop=True)
            h_t = call[:, 2 * P:2 * P + 1]
            bias = []
            for i in range(2):
                nc.tensor.matmul(bias_ps[:, i:i + 1],
                                 lhsT=call[:, i * P:(i + 1) * P],
                                 rhs=h_t, start=True, stop=True)
                bias_sb = persist.tile([P, 1], f32, name=f"bias{i}",
                                       tag=f"bias{i}")
                nc.vector.tensor_copy(bias_sb, bias_ps[:, i:i + 1])
                bias.append(bias_sb)

            # ---- conv stacks ----------------------------------------------
            drain_rr = [0]

            def conv_layer(src, layer, br, dst=None, fuse=None):
                d = 2 ** layer
                base = 2 * SCHED.index((br, layer))
                w0r = wall[:, base * P:(base + 1) * P]
                w1r = wall[:, (base + 1) * P:(base + 2) * P]
                for j in range(NT):
                    t0 = j * NTAP_TILE
                    # d < 512 boundary reads dip into the zero margin;
                    # d >= 512 boundaries are tile-aligned -> tap 0 skipped.
                    has_tap0 = t0 + NTAP_TILE > d
                    ps = mpsum.tile([P, NTAP_TILE], f32,
                                    name=f"ps_{br}{layer}_{j}", tag="mp")
                    nc.tensor.matmul(
                        ps, lhsT=w1r,
                        rhs=src[:, MARGIN + t0:MARGIN + t0 + NTAP_TILE],
                        start=True, stop=not has_tap0)
                    if has_tap0:
                        o0 = MARGIN + t0 - d
                        nc.tensor.matmul(
                            ps, lhsT=w0r, rhs=src[:, o0:o0 + NTAP_TILE],
                            start=False, stop=True)
                    if fuse is None:
                        dslice = dst[:, MARGIN + t0:MARGIN + t0 + NTAP_TILE]
                        # alternate drains DVE/ACT so neither engine gates PE
                        if drain_rr[0] % 2 == 1:
                            nc.scalar.copy(dslice, ps)
                        else:
                            nc.vector.tensor_copy(dslice, ps)
                        drain_rr[0] += 1
                    else:
                        fuse(j, ps)

            # f-l12's drain IS the tanh (ScalarE, fused bias) straight into
            # scratch (g-l0's parked output is dead once g-l1 has read it),
            # keeping the final g-layer's epilogue off the ACT critical path.
            def tanh_drain(j, ps_f):
                t0 = MARGIN + j * NTAP_TILE
                nc.scalar.activation(scratch[:, t0:t0 + NTAP_TILE], ps_f,
                                     AF.Tanh, bias=bias[0])

            # g-l12 fused with the gating epilogue
            def epilogue(j, ps_g):
                t0 = MARGIN + j * NTAP_TILE
                sig = epool.tile([P, NTAP_TILE], f32, name=f"sig{j}", tag="sig")
                nc.scalar.activation(sig, ps_g, AF.Sigmoid, bias=bias[1])
                # scratch[j] holds tanh(f); the gated output overwrites it
                # and is DMA'd out from there. Read via the f32 view (fp32r
                # bits are valid rounded-fp32 bits); write stays f32r-typed.
                nc.vector.tensor_mul(scratch[:, t0:t0 + NTAP_TILE],
                                     scratch32[:, t0:t0 + NTAP_TILE], sig)

            # buffer rotation per SCHED:
            #   f: x0 -> A -> B -> A ... (f-l12 reads B, tanh -> scratch)
            #   g: x0 -> scratch -> A -> B ... (g-l12 reads A, epilogue)
            cur = {"f": x0, "g": x0}
            for br, layer in SCHED:
                if (br, layer) == ("f", 12):
                    conv_layer(cur["f"], layer, br, fuse=tanh_drain)
                elif (br, layer) == ("g", 12):
                    conv_layer(cur["g"], layer, br, fuse=epilogue)
                else:
                    if br == "f":
                        dst = bufA if layer % 2 == 0 else bufB
                    else:
                        dst = scratch if layer == 0 else \
                            (bufA if layer % 2 == 1 else bufB)
                    conv_layer(cur[br], layer, br, dst=dst)
                    cur[br] = dst

            # ---- store output (channel-major; host restores [b,t,c]) ------
            for q in range(NQ):
                nc.sync.dma_start(
                    out=outs[q],
                    in_=scratch32[:, MARGIN + q * QT:MARGIN + (q + 1) * QT])

    nc.compile()
    return nc


def get_program():
    if "nc" not in _PROGRAM_CACHE:
        _PROGRAM_CACHE["nc"] = _build_program()
    return _PROGRAM_CACHE["nc"]


def make_in_maps(x, h, Wf, Wg, Vf, Vg):
    x = np.asarray(x, dtype=np.float32)
    h = np.asarray(h, dtype=np.float32)
    eye2 = np.eye(2, dtype=np.float32)
    # SCHED-ordered [branch, layer, tap] -> kron(I2, W[tap]) as lhsT
    # [K=(b,cin), M=(b,cout)]
    Wn = {"f": np.asarray(Wf, dtype=np.float32),
          "g": np.asarray(Wg, dtype=np.float32)}
    wpack = np.zeros((NW, P, P), dtype=np.float32)
    for pos, (br, layer) in enumerate(SCHED):
        for tap in range(K):
            wpack[2 * pos + tap] = np.kron(eye2, Wn[br][layer, tap])
    # wall[p, i*P + m] = wpack[i, p, m], pre-rounded to fp32r bits
    wcols = fp32r_bits(wpack.transpose(1, 0, 2).reshape(P, NW * P))
    vcat = np.concatenate(
        [np.kron(eye2, np.asarray(V, dtype=np.float32)) for V in (Vf, Vg)],
        axis=1)  # [128, 256]

    in_maps = []
    for core in range(N_CORES):
        sl = slice(core * BPC, (core + 1) * BPC)
        xcm = fp32r_bits(x[sl].transpose(0, 2, 1).reshape(P, T))  # [(b,c), t]
        consts = np.ascontiguousarray(
            np.concatenate([vcat, h[sl].reshape(P, 1)], axis=1))
        in_maps.append({"consts": consts, "xr": xcm, "wr": wcols})
    return in_maps


def assemble_output(results):
    full = np.empty((B, T, C), dtype=np.float32)
    for core, r in enumerate(results):
        cm = np.concatenate([r[f"out_q{q}"] for q in range(NQ)], axis=1)
        full[core * BPC:(core + 1) * BPC] = \
            cm.reshape(BPC, C, T).transpose(0, 2, 1)
    return full


def kernel(x, h, Wf, Wg, Vf, Vg):
    from concourse import bass_utils

    nc = get_program()
    in_maps = make_in_maps(x, h, Wf, Wg, Vf, Vg)
    res = bass_utils.run_bass_kernel_spmd(nc, in_maps,
                                          core_ids=list(range(N_CORES)))
    return assemble_output(res.results)
